# revision 27
# baseline (speedup 1.0000x reference)
"""Trainium2 Bass kernel for a single-head causal attention block.

Problem (hardcoded):
  input_val: [4, 4096, 1024] f32, Wq/Wk/Wv: [64, 1024] f32, k_mask: [4, 4096] i32
  out = softmax(causal_mask(QK^T/sqrt(64))) @ V  -> [4, 4096, 64] f32

Sharding: 8 cores = 4 batches x 2 roles. Within a batch, queries are split
into 16 chunks of 256; role r takes global chunks {2j+r}. Every core runs an
IDENTICAL program (SPMD); the chunk identity is carried entirely by the
per-core input data (xq row selection, causal-mask tile contents).

Device program per core (matmul contraction operands at partition base 0 -
base-64 operands crash this runtime):
  - K^T/V^T projection: lhsT = [Wk^T|Wv^T] (c-chunked), rhs = X^T -> PSUM
    [K^T rows 0:64 | V^T rows 64:128, 512] per t-block; DVE copies K^T to
    SBUF KT[0:64] and V^T (partition-shifted) to VT0[0:64], both bf16.
  - V1[kb] = [V | ones] built via PE-mode transpose of VT0 128-col blocks
    (bf16 PSUM out) + DVE copy; ones column memset once.
  - Q^T projection from xq (own queries, locally contiguous).
  - attention in S^T layout over local chunk-PAIRS m (512 queries), flash
    style; k-blocks (kb) of 128 keys, absolute range 0..8m+7:
      shared kbs 0..8m+3 (needed by both chunks): S^T [128,512] = K^T.T @ Q^T
        (K=64), batched per kb-pair into one [128,2,512] PSUM tile, one ACT
        exp (N=1024), causal mask (DVE) on the last two pairs, then per kb
        one O^T matmul [65,512] += V1[kb].T @ P^T (65th col = ones gives the
        softmax denominator for free).
      tail kbs 8m+4..8m+7 (second chunk only): same, N=256, accumulating
        into oacc[:, 256:512].
  - out [65, 2048] f32; host divides rows 0:64 by row 64 and scatters.
"""
import numpy as np

B, T, C, H = 4, 4096, 1024, 64
N_CORES = 8
CC = 8          # contraction chunks of 128 over C
TB = 8          # key t-blocks of 512
NKB = 32        # key blocks of 128
NPAIR = 4       # local chunk-pairs of 512 queries
SCALE = 1.0 / np.sqrt(H)

_CACHE = {}


def _build_program(use_kmask: bool):
    from contextlib import ExitStack
    import concourse.tile as tile
    from concourse import bacc, mybir

    BF16 = mybir.dt.bfloat16
    F32 = mybir.dt.float32
    Exp = mybir.ActivationFunctionType.Exp

    nc = bacc.Bacc("TRN2", target_bir_lowering=False, debug=False)
    xt = nc.dram_tensor("xt", [TB, 128, CC, 512], BF16, kind="ExternalInput")
    xq = nc.dram_tensor("xq", [NPAIR, 128, CC, 512], BF16, kind="ExternalInput")
    wkv = nc.dram_tensor("wkv", [128, CC, 128], BF16, kind="ExternalInput")
    wq = nc.dram_tensor("wq", [128, CC, 64], BF16, kind="ExternalInput")
    cm2 = nc.dram_tensor("cm2", [128, 4, 512], BF16, kind="ExternalInput")
    cm1 = nc.dram_tensor("cm1", [128, 2, 512], BF16, kind="ExternalInput")
    i64 = nc.dram_tensor("i64", [64, 64], BF16, kind="ExternalInput")
    if use_kmask:
        km = nc.dram_tensor("km", [128, NKB], F32, kind="ExternalInput")
    o = nc.dram_tensor("o", [65, 2048], F32, kind="ExternalOutput")

    with tile.TileContext(nc) as tc:
        with ExitStack() as ctx:
            const = ctx.enter_context(tc.tile_pool(name="const", bufs=1))
            ppool = ctx.enter_context(tc.tile_pool(name="ptp", bufs=4))
            projp = ctx.enter_context(tc.tile_pool(name="projp", bufs=1, space="PSUM"))
            vtp = ctx.enter_context(tc.tile_pool(name="vtp", bufs=1, space="PSUM"))
            stp = ctx.enter_context(tc.tile_pool(name="stp", bufs=2, space="PSUM"))
            otp = ctx.enter_context(tc.tile_pool(name="otp", bufs=2, space="PSUM"))

            XT = const.tile([128, TB, CC, 512], BF16)
            XQ = const.tile([128, NPAIR, CC, 512], BF16)
            WKV = const.tile([128, CC, 128], BF16)
            WQ = const.tile([128, CC, 64], BF16)
            CM2 = const.tile([128, 4, 512], BF16)
            CM1 = const.tile([128, 2, 512], BF16)
            ID = const.tile([64, 64], BF16)
            KT = const.tile([64, T], BF16)
            QT = const.tile([64, 2048], BF16)
            VT0 = const.tile([64, T], BF16)
            V1 = const.tile([128, NKB, 80], BF16)  # [:, kb, 0:64]=V, col 64=1
            OUT = const.tile([65, 2048], F32)
            if use_kmask:
                KM = const.tile([128, NKB], F32)
                nc.sync.dma_start(KM[:], km.ap())

            nc.sync.dma_start(WQ[:], wq.ap())
            nc.sync.dma_start(WKV[:], wkv.ap())
            nc.sync.dma_start(ID[:], i64.ap())
            nc.sync.dma_start(CM2[:], cm2.ap())
            nc.sync.dma_start(CM1[:], cm1.ap())
            nc.gpsimd.dma_start(XQ[:, 0, 0:4], xq.ap()[0][:, 0:4])
            nc.gpsimd.dma_start(XQ[:, 0, 4:8], xq.ap()[0][:, 4:8])
            nc.gpsimd.memset(V1[:, :, 64:65], 1.0)

            def emit_kv(tb, split=False):
                if split:
                    nc.gpsimd.dma_start(XT[:, tb, 0:4], xt.ap()[tb][:, 0:4])
                    nc.gpsimd.dma_start(XT[:, tb, 4:8], xt.ap()[tb][:, 4:8])
                else:
                    nc.gpsimd.dma_start(XT[:, tb], xt.ap()[tb])
                ps = projp.tile([128, 512], F32, name="ps", tag="ps")
                for cc in range(CC):
                    nc.tensor.matmul(ps[:], WKV[:, cc, :], XT[:, tb, cc, :],
                                     start=(cc == 0), stop=(cc == CC - 1))
                sl = slice(512 * tb, 512 * (tb + 1))
                nc.vector.tensor_copy(KT[:, sl], ps[0:64, :])
                nc.vector.tensor_copy(VT0[0:64, sl], ps[64:128, :])

            def emit_trans(tb):
                vt_ps = vtp.tile([128, 4, 64], BF16, name="vt_ps", tag="vt")
                for u in range(4):
                    kb = 4 * tb + u
                    nc.tensor.transpose(vt_ps[:, u, :],
                                        VT0[:, 128 * kb:128 * (kb + 1)], ID[:])
                nc.vector.tensor_copy(V1[:, 4 * tb:4 * tb + 4, 0:64], vt_ps[:])

            def emit_q(m):
                ps = projp.tile([128, 512], F32, name="ps", tag="ps")
                for cc in range(CC):
                    nc.tensor.matmul(ps[0:64, :], WQ[:, cc, :], XQ[:, m, cc, :],
                                     start=(cc == 0), stop=(cc == CC - 1))
                nc.vector.tensor_copy(QT[:, 512 * m:512 * (m + 1)], ps[0:64, :])

            def emit_kv2_and_prefetch(m):
                emit_kv(2 * m + 1)
                if m + 1 < NPAIR:
                    nc.gpsimd.dma_start(XQ[:, m + 1], xq.ap()[m + 1])

            def kmul(pt_slice, kb):
                nc.vector.tensor_scalar_mul(pt_slice, pt_slice, KM[:, kb:kb + 1])

            def emit_S_shared(m, sp):
                st = stp.tile([128, 2, 512], F32, name="st", tag="st")
                qsl = slice(512 * m, 512 * (m + 1))
                for u in range(2):
                    kb = 2 * sp + u
                    nc.tensor.matmul(st[:, u, :], KT[:, 128 * kb:128 * (kb + 1)],
                                     QT[:, qsl], start=True, stop=True)
                pt = ppool.tile([128, 2, 512], BF16, name="pt", tag="pt")
                nc.scalar.activation(pt[:], st[:], Exp, scale=SCALE)
                if sp >= 4 * m:
                    t0 = 2 * (sp - 4 * m)
                    nc.vector.tensor_mul(pt[:], pt[:], CM2[:, t0:t0 + 2, :])
                if use_kmask:
                    for u in range(2):
                        kmul(pt[:, u, :], 2 * sp + u)
                return pt

            def emit_O_shared(m, sp, pt, oacc):
                for u in range(2):
                    kb = 2 * sp + u
                    nc.tensor.matmul(oacc[:], V1[:, kb, 0:65], pt[:, u, :],
                                     start=(kb == 0), stop=False)


            def emit_S_tail(m):
                st = stp.tile([128, 2, 512], F32, name="st", tag="st")
                qsl = slice(512 * m + 256, 512 * (m + 1))
                for t in range(4):
                    kb = 8 * m + 4 + t
                    nc.tensor.matmul(
                        st[:, t // 2, 256 * (t % 2):256 * (t % 2) + 256],
                        KT[:, 128 * kb:128 * (kb + 1)],
                        QT[:, qsl], start=True, stop=True)
                pt = ppool.tile([128, 2, 512], BF16, name="pt", tag="pt")
                nc.scalar.activation(pt[:], st[:], Exp, scale=SCALE)
                nc.vector.tensor_mul(pt[:], pt[:], CM1[:])
                if use_kmask:
                    for t in range(4):
                        kmul(pt[:, t // 2, 256 * (t % 2):256 * (t % 2) + 256],
                             8 * m + 4 + t)
                return pt

            def emit_O_tail(m, pt, oacc):
                for t in range(4):
                    kb = 8 * m + 4 + t
                    nc.tensor.matmul(
                        oacc[:, 256:512], V1[:, kb, 0:65],
                        pt[:, t // 2, 256 * (t % 2):256 * (t % 2) + 256],
                        start=False, stop=(t == 3))
                nc.vector.tensor_copy(OUT[:, 512 * m:512 * (m + 1)], oacc[:])
                nc.sync.dma_start(o.ap()[:, 512 * m:512 * (m + 1)],
                                  OUT[:, 512 * m:512 * (m + 1)])

            # skewed emission: the O-matmuls of work item i are emitted two
            # S-slots later, keeping PE fed during exp latency. PSUM
            # accumulation is element-wise so O order within a pair only
            # needs the start=True matmul (kb==0) first, which the FIFO
            # preserves.
            pending = []  # [(kind, m, sp, pt), ...]
            oaccs = {}

            def flush_one():
                kind, m_, sp_, pt_ = pending.pop(0)
                if kind == "shared":
                    emit_O_shared(m_, sp_, pt_, oaccs[m_])
                else:
                    emit_O_tail(m_, pt_, oaccs[m_])

            def flush_pending(depth=2):
                while len(pending) > depth:
                    flush_one()

            # Per pair m, interleave infra (projections, V-transposes) with
            # the attention slots so the PE always has queued work while the
            # DVE drains projection PSUM: q first (its copy completes under
            # the KV matmuls), then alternate kv/trans emissions with S-slots
            # that depend only on earlier t-blocks.
            for m in range(NPAIR):
                oaccs[m] = otp.tile([65, 512], F32, name="oacc", tag="oacc")
                infra = [lambda m=m: emit_q(m),
                         lambda m=m: emit_kv(2 * m, split=(m == 0)),
                         None,  # S(0) goes here
                         lambda m=m: emit_trans(2 * m),
                         None,  # S(1) goes here
                         lambda m=m: emit_kv2_and_prefetch(m),
                         None,  # S(2)
                         lambda m=m: emit_trans(2 * m + 1)]
                infra[0](); infra[1]()
                sp = 0
                nslots = 4 * m + 2
                # emit S(0); then alternate remaining infra with S-slots
                pt = emit_S_shared(m, 0)
                pending.append(("shared", m, 0, pt))
                flush_pending()
                for step in (3, 5, 7):
                    infra[step]()
                    if sp + 1 < nslots:
                        sp += 1
                        pt = emit_S_shared(m, sp)
                        pending.append(("shared", m, sp, pt))
                        flush_pending()
                while sp + 1 < nslots:
                    sp += 1
                    pt = emit_S_shared(m, sp)
                    pending.append(("shared", m, sp, pt))
                    flush_pending()
                pt = emit_S_tail(m)
                pending.append(("tail", m, None, pt))
                flush_pending()
            flush_pending(depth=0)

    nc.compile()
    return nc


def _get_program(use_kmask: bool):
    key = ("prog", use_kmask)
    if key not in _CACHE:
        _CACHE[key] = _build_program(use_kmask)
    return _CACHE[key]


def _host_prep(input_val, Wq, Wk, Wv, k_mask, use_kmask):
    import ml_dtypes
    bf = ml_dtypes.bfloat16

    wkv_np = np.empty((128, CC, 128), dtype=bf)
    wkv_np[:, :, 0:64] = Wk.reshape(64, CC, 128).transpose(2, 1, 0).astype(bf)
    wkv_np[:, :, 64:128] = Wv.reshape(64, CC, 128).transpose(2, 1, 0).astype(bf)
    wq_np = Wq.reshape(64, CC, 128).transpose(2, 1, 0).astype(bf).copy()
    id_np = np.eye(64, dtype=np.float32).astype(bf)

    kk = np.arange(128)[:, None]
    qq = np.arange(256)[None, :]
    cm2s, cm1s = [], []
    for r in range(2):
        c1 = np.empty((128, 4, 256), dtype=bf)
        for t in range(4):
            c1[:, t, :] = (kk <= 256 * r - 128 * t + qq).astype(bf)
        cm1s.append(np.ascontiguousarray(c1).reshape(128, 2, 512).copy())
        c2 = np.ones((128, 4, 512), dtype=bf)
        c2[:, :, 0:256] = c1
        cm2s.append(c2)

    xts = []
    for b in range(B):
        Xb = np.asarray(input_val[b], dtype=np.float32)
        xts.append(np.ascontiguousarray(
            Xb.reshape(TB, 512, CC, 128).transpose(0, 3, 2, 1)).astype(bf))

    in_maps = []
    for c in range(N_CORES):
        b, r = c // 2, c % 2
        Xb = np.asarray(input_val[b], dtype=np.float32)
        rows = np.concatenate(
            [np.arange(256 * (2 * j + r), 256 * (2 * j + r) + 256)
             for j in range(2 * NPAIR)])
        Xqrows = Xb[rows]  # [2048, 1024]
        xq_np = np.ascontiguousarray(
            Xqrows.reshape(NPAIR, 512, CC, 128).transpose(0, 3, 2, 1)).astype(bf)
        m = {"xt": xts[b], "xq": xq_np, "wkv": wkv_np, "wq": wq_np,
             "cm2": cm2s[r], "cm1": cm1s[r], "i64": id_np}
        if use_kmask:
            m["km"] = np.ascontiguousarray(np.asarray(
                k_mask[b], dtype=np.float32).reshape(NKB, 128).T)
        in_maps.append(m)
    return in_maps


def _unshard(results):
    out = np.empty((B, T, H), dtype=np.float32)
    for c in range(N_CORES):
        b, r = c // 2, c % 2
        on = results[c]["o"]
        num = on[0:64, :]
        den = on[64, :]
        for j in range(2 * NPAIR):
            g = 2 * j + r
            blk = num[:, 256 * j:256 * (j + 1)] / den[None, 256 * j:256 * (j + 1)]
            out[b, 256 * g:256 * (g + 1), :] = blk.T
    return out


def kernel(input_val, Wq, Wk, Wv, k_mask):
    import concourse.bass_utils as bu

    input_val = np.asarray(input_val)
    Wq, Wk, Wv = (np.asarray(a, dtype=np.float32) for a in (Wq, Wk, Wv))
    k_mask = np.asarray(k_mask)
    use_kmask = not bool(np.all(k_mask == 1))

    nc = _get_program(use_kmask)
    in_maps = _host_prep(input_val, Wq, Wk, Wv, k_mask, use_kmask)
    res = bu.run_bass_kernel_spmd(nc, in_maps, core_ids=list(range(N_CORES)))
    return _unshard(res.results)


def kernel_traced(input_val, Wq, Wk, Wv, k_mask, **trace_kwargs):
    """Like kernel() but returns (output, BassKernelResults) with tracing on."""
    import concourse.bass_utils as bu

    input_val = np.asarray(input_val)
    k_mask = np.asarray(k_mask)
    use_kmask = not bool(np.all(k_mask == 1))
    nc = _get_program(use_kmask)
    in_maps = _host_prep(input_val, np.asarray(Wq, dtype=np.float32),
                         np.asarray(Wk, dtype=np.float32),
                         np.asarray(Wv, dtype=np.float32), k_mask, use_kmask)
    res = bu.run_bass_kernel_spmd(nc, in_maps, core_ids=list(range(N_CORES)),
                                  trace=True, **trace_kwargs)
    return _unshard(res.results), res


# revision 28
# speedup vs baseline: 1.0268x; 1.0268x over previous
"""Trainium2 Bass kernel for a single-head causal attention block.

Problem (hardcoded):
  input_val: [4, 4096, 1024] f32, Wq/Wk/Wv: [64, 1024] f32, k_mask: [4, 4096] i32
  out = softmax(causal_mask(QK^T/sqrt(64))) @ V  -> [4, 4096, 64] f32

Sharding: 8 cores = 4 batches x 2 roles. Within a batch, queries are split
into 16 chunks of 256; role r takes global chunks {2j+r}. Every core runs an
IDENTICAL program (SPMD); the chunk identity is carried entirely by the
per-core input data (xq row selection, causal-mask tile contents).

Device program per core (matmul contraction operands at partition base 0 -
base-64 operands crash this runtime):
  - K^T/V^T projection: lhsT = [Wk^T|Wv^T] (c-chunked), rhs = X^T -> PSUM
    [K^T rows 0:64 | V^T rows 64:128, 512] per t-block; DVE copies K^T to
    SBUF KT[0:64] and V^T (partition-shifted) to VT0[0:64], both bf16.
  - V1[kb] = [V | ones] built via PE-mode transpose of VT0 128-col blocks
    (bf16 PSUM out) + DVE copy; ones column memset once.
  - Q^T projection from xq (own queries, locally contiguous).
  - attention in S^T layout over local chunk-PAIRS m (512 queries), flash
    style; k-blocks (kb) of 128 keys, absolute range 0..8m+7:
      shared kbs 0..8m+3 (needed by both chunks): S^T [128,512] = K^T.T @ Q^T
        (K=64), batched per kb-pair into one [128,2,512] PSUM tile, one ACT
        exp (N=1024), causal mask (DVE) on the last two pairs, then per kb
        one O^T matmul [65,512] += V1[kb].T @ P^T (65th col = ones gives the
        softmax denominator for free).
      tail kbs 8m+4..8m+7 (second chunk only): same, N=256, accumulating
        into oacc[:, 256:512].
  - out [65, 2048] f32; host divides rows 0:64 by row 64 and scatters.
"""
import numpy as np

B, T, C, H = 4, 4096, 1024, 64
N_CORES = 8
CC = 8          # contraction chunks of 128 over C
TB = 8          # key t-blocks of 512
NKB = 32        # key blocks of 128
NPAIR = 4       # local chunk-pairs of 512 queries
SCALE = 1.0 / np.sqrt(H)

_CACHE = {}


def _build_program(use_kmask: bool):
    from contextlib import ExitStack
    import concourse.tile as tile
    from concourse import bacc, mybir

    BF16 = mybir.dt.bfloat16
    F32 = mybir.dt.float32
    Exp = mybir.ActivationFunctionType.Exp

    nc = bacc.Bacc("TRN2", target_bir_lowering=False, debug=False)
    xt = nc.dram_tensor("xt", [TB, 128, CC, 512], BF16, kind="ExternalInput")
    xq = nc.dram_tensor("xq", [NPAIR, 128, CC, 512], BF16, kind="ExternalInput")
    wkv = nc.dram_tensor("wkv", [128, CC, 128], BF16, kind="ExternalInput")
    wq = nc.dram_tensor("wq", [128, CC, 64], BF16, kind="ExternalInput")
    cm2 = nc.dram_tensor("cm2", [128, 4, 512], BF16, kind="ExternalInput")
    cm1 = nc.dram_tensor("cm1", [128, 2, 512], BF16, kind="ExternalInput")
    i64 = nc.dram_tensor("i64", [64, 64], BF16, kind="ExternalInput")
    if use_kmask:
        km = nc.dram_tensor("km", [128, NKB], F32, kind="ExternalInput")
    o = nc.dram_tensor("o", [65, 2048], F32, kind="ExternalOutput")

    with tile.TileContext(nc) as tc:
        with ExitStack() as ctx:
            const = ctx.enter_context(tc.tile_pool(name="const", bufs=1))
            ppool = ctx.enter_context(tc.tile_pool(name="ptp", bufs=4))
            projp = ctx.enter_context(tc.tile_pool(name="projp", bufs=1, space="PSUM"))
            vtp = ctx.enter_context(tc.tile_pool(name="vtp", bufs=1, space="PSUM"))
            stp = ctx.enter_context(tc.tile_pool(name="stp", bufs=2, space="PSUM"))
            otp = ctx.enter_context(tc.tile_pool(name="otp", bufs=2, space="PSUM"))

            XT = const.tile([128, TB, CC, 512], BF16)
            XQ = const.tile([128, NPAIR, CC, 512], BF16)
            WKV = const.tile([128, CC, 128], BF16)
            WQ = const.tile([128, CC, 64], BF16)
            CM2 = const.tile([128, 4, 512], BF16)
            CM1 = const.tile([128, 2, 512], BF16)
            ID = const.tile([64, 64], BF16)
            KT = const.tile([64, T], BF16)
            QT = const.tile([64, 2048], BF16)
            VT0 = const.tile([64, T], BF16)
            V1 = const.tile([128, NKB, 80], BF16)  # [:, kb, 0:64]=V, col 64=1
            OUT = const.tile([65, 2048], F32)
            if use_kmask:
                KM = const.tile([128, NKB], F32)
                nc.sync.dma_start(KM[:], km.ap())

            nc.sync.dma_start(WQ[:], wq.ap())
            nc.sync.dma_start(WKV[:], wkv.ap())
            nc.sync.dma_start(ID[:], i64.ap())
            nc.sync.dma_start(CM2[:], cm2.ap())
            nc.sync.dma_start(CM1[:], cm1.ap())
            nc.gpsimd.dma_start(XQ[:, 0], xq.ap()[0])
            nc.gpsimd.memset(V1[:, :, 64:65], 1.0)

            def emit_kv(tb, split=False):
                if split:
                    nc.gpsimd.dma_start(XT[:, tb, 0:4], xt.ap()[tb][:, 0:4])
                    nc.gpsimd.dma_start(XT[:, tb, 4:8], xt.ap()[tb][:, 4:8])
                else:
                    nc.gpsimd.dma_start(XT[:, tb], xt.ap()[tb])
                ps = projp.tile([128, 512], F32, name="ps", tag="ps")
                for cc in range(CC):
                    nc.tensor.matmul(ps[:], WKV[:, cc, :], XT[:, tb, cc, :],
                                     start=(cc == 0), stop=(cc == CC - 1))
                sl = slice(512 * tb, 512 * (tb + 1))
                nc.vector.tensor_copy(KT[:, sl], ps[0:64, :])
                nc.vector.tensor_copy(VT0[0:64, sl], ps[64:128, :])

            def emit_trans(tb):
                vt_ps = vtp.tile([128, 4, 64], BF16, name="vt_ps", tag="vt")
                for u in range(4):
                    kb = 4 * tb + u
                    nc.tensor.transpose(vt_ps[:, u, :],
                                        VT0[:, 128 * kb:128 * (kb + 1)], ID[:])
                nc.vector.tensor_copy(V1[:, 4 * tb:4 * tb + 4, 0:64], vt_ps[:])

            def emit_q(m):
                ps = projp.tile([128, 512], F32, name="ps", tag="ps")
                for cc in range(CC):
                    nc.tensor.matmul(ps[0:64, :], WQ[:, cc, :], XQ[:, m, cc, :],
                                     start=(cc == 0), stop=(cc == CC - 1))
                nc.vector.tensor_copy(QT[:, 512 * m:512 * (m + 1)], ps[0:64, :])

            def emit_kv2_and_prefetch(m):
                emit_kv(2 * m + 1)
                if m + 1 < NPAIR:
                    nc.gpsimd.dma_start(XQ[:, m + 1], xq.ap()[m + 1])

            def kmul(pt_slice, kb):
                nc.vector.tensor_scalar_mul(pt_slice, pt_slice, KM[:, kb:kb + 1])

            def emit_S_shared(m, sp):
                st = stp.tile([128, 2, 512], F32, name="st", tag="st")
                qsl = slice(512 * m, 512 * (m + 1))
                for u in range(2):
                    kb = 2 * sp + u
                    nc.tensor.matmul(st[:, u, :], KT[:, 128 * kb:128 * (kb + 1)],
                                     QT[:, qsl], start=True, stop=True)
                pt = ppool.tile([128, 2, 512], BF16, name="pt", tag="pt")
                nc.scalar.activation(pt[:], st[:], Exp, scale=SCALE)
                if sp >= 4 * m:
                    t0 = 2 * (sp - 4 * m)
                    nc.vector.tensor_mul(pt[:], pt[:], CM2[:, t0:t0 + 2, :])
                if use_kmask:
                    for u in range(2):
                        kmul(pt[:, u, :], 2 * sp + u)
                return pt

            def emit_O_shared(m, sp, pt, oacc):
                for u in range(2):
                    kb = 2 * sp + u
                    nc.tensor.matmul(oacc[:], V1[:, kb, 0:65], pt[:, u, :],
                                     start=(kb == 0), stop=False)

            def emit_S_tail(m):
                st = stp.tile([128, 2, 512], F32, name="st", tag="st")
                qsl = slice(512 * m + 256, 512 * (m + 1))
                for t in range(4):
                    kb = 8 * m + 4 + t
                    nc.tensor.matmul(
                        st[:, t // 2, 256 * (t % 2):256 * (t % 2) + 256],
                        KT[:, 128 * kb:128 * (kb + 1)],
                        QT[:, qsl], start=True, stop=True)
                pt = ppool.tile([128, 2, 512], BF16, name="pt", tag="pt")
                nc.scalar.activation(pt[:], st[:], Exp, scale=SCALE)
                nc.vector.tensor_mul(pt[:], pt[:], CM1[:])
                if use_kmask:
                    for t in range(4):
                        kmul(pt[:, t // 2, 256 * (t % 2):256 * (t % 2) + 256],
                             8 * m + 4 + t)
                return pt

            def emit_O_tail(m, pt, oacc):
                for t in range(4):
                    kb = 8 * m + 4 + t
                    nc.tensor.matmul(
                        oacc[:, 256:512], V1[:, kb, 0:65],
                        pt[:, t // 2, 256 * (t % 2):256 * (t % 2) + 256],
                        start=False, stop=(t == 3))
                nc.vector.tensor_copy(OUT[:, 512 * m:512 * (m + 1)], oacc[:])
                nc.sync.dma_start(o.ap()[:, 512 * m:512 * (m + 1)],
                                  OUT[:, 512 * m:512 * (m + 1)])

            # skewed emission: the O-matmuls of work item i are emitted two
            # S-slots later, keeping PE fed during exp latency. PSUM
            # accumulation is element-wise so O order within a pair only
            # needs the start=True matmul (kb==0) first, which the FIFO
            # preserves.
            pending = []  # [(kind, m, sp, pt), ...]
            oaccs = {}

            def flush_one():
                kind, m_, sp_, pt_ = pending.pop(0)
                if kind == "shared":
                    emit_O_shared(m_, sp_, pt_, oaccs[m_])
                else:
                    emit_O_tail(m_, pt_, oaccs[m_])

            def flush_pending(depth=2):
                while len(pending) > depth:
                    flush_one()

            # Per pair m, interleave infra (projections, V-transposes) with
            # the attention slots so the PE always has queued work while the
            # DVE drains projection PSUM: q first (its copy completes under
            # the KV matmuls), then alternate kv/trans emissions with S-slots
            # that depend only on earlier t-blocks.
            for m in range(NPAIR):
                oaccs[m] = otp.tile([65, 512], F32, name="oacc", tag="oacc")
                infra = [lambda m=m: emit_q(m),
                         lambda m=m: emit_kv(2 * m),
                         None,  # S(0) goes here
                         lambda m=m: emit_trans(2 * m),
                         None,  # S(1) goes here
                         lambda m=m: emit_kv2_and_prefetch(m),
                         None,  # S(2)
                         lambda m=m: emit_trans(2 * m + 1)]
                infra[0](); infra[1]()
                sp = 0
                nslots = 4 * m + 2
                # emit S(0); then alternate remaining infra with S-slots
                pt = emit_S_shared(m, 0)
                pending.append(("shared", m, 0, pt))
                flush_pending()
                for step in (3, 5, 7):
                    infra[step]()
                    if sp + 1 < nslots:
                        sp += 1
                        pt = emit_S_shared(m, sp)
                        pending.append(("shared", m, sp, pt))
                        flush_pending()
                while sp + 1 < nslots:
                    sp += 1
                    pt = emit_S_shared(m, sp)
                    pending.append(("shared", m, sp, pt))
                    flush_pending()
                pt = emit_S_tail(m)
                pending.append(("tail", m, None, pt))
                flush_pending()
            flush_pending(depth=0)

    nc.compile()
    return nc


def _get_program(use_kmask: bool):
    key = ("prog", use_kmask)
    if key not in _CACHE:
        _CACHE[key] = _build_program(use_kmask)
    return _CACHE[key]


def _host_prep(input_val, Wq, Wk, Wv, k_mask, use_kmask):
    import ml_dtypes
    bf = ml_dtypes.bfloat16

    wkv_np = np.empty((128, CC, 128), dtype=bf)
    wkv_np[:, :, 0:64] = Wk.reshape(64, CC, 128).transpose(2, 1, 0).astype(bf)
    wkv_np[:, :, 64:128] = Wv.reshape(64, CC, 128).transpose(2, 1, 0).astype(bf)
    wq_np = Wq.reshape(64, CC, 128).transpose(2, 1, 0).astype(bf).copy()
    id_np = np.eye(64, dtype=np.float32).astype(bf)

    kk = np.arange(128)[:, None]
    qq = np.arange(256)[None, :]
    cm2s, cm1s = [], []
    for r in range(2):
        c1 = np.empty((128, 4, 256), dtype=bf)
        for t in range(4):
            c1[:, t, :] = (kk <= 256 * r - 128 * t + qq).astype(bf)
        cm1s.append(np.ascontiguousarray(c1).reshape(128, 2, 512).copy())
        c2 = np.ones((128, 4, 512), dtype=bf)
        c2[:, :, 0:256] = c1
        cm2s.append(c2)

    xts = []
    for b in range(B):
        Xb = np.asarray(input_val[b], dtype=np.float32)
        xts.append(np.ascontiguousarray(
            Xb.reshape(TB, 512, CC, 128).transpose(0, 3, 2, 1)).astype(bf))

    in_maps = []
    for c in range(N_CORES):
        b, r = c // 2, c % 2
        Xb = np.asarray(input_val[b], dtype=np.float32)
        rows = np.concatenate(
            [np.arange(256 * (2 * j + r), 256 * (2 * j + r) + 256)
             for j in range(2 * NPAIR)])
        Xqrows = Xb[rows]  # [2048, 1024]
        xq_np = np.ascontiguousarray(
            Xqrows.reshape(NPAIR, 512, CC, 128).transpose(0, 3, 2, 1)).astype(bf)
        m = {"xt": xts[b], "xq": xq_np, "wkv": wkv_np, "wq": wq_np,
             "cm2": cm2s[r], "cm1": cm1s[r], "i64": id_np}
        if use_kmask:
            m["km"] = np.ascontiguousarray(np.asarray(
                k_mask[b], dtype=np.float32).reshape(NKB, 128).T)
        in_maps.append(m)
    return in_maps


def _unshard(results):
    out = np.empty((B, T, H), dtype=np.float32)
    for c in range(N_CORES):
        b, r = c // 2, c % 2
        on = results[c]["o"]
        num = on[0:64, :]
        den = on[64, :]
        for j in range(2 * NPAIR):
            g = 2 * j + r
            blk = num[:, 256 * j:256 * (j + 1)] / den[None, 256 * j:256 * (j + 1)]
            out[b, 256 * g:256 * (g + 1), :] = blk.T
    return out


def kernel(input_val, Wq, Wk, Wv, k_mask):
    import concourse.bass_utils as bu

    input_val = np.asarray(input_val)
    Wq, Wk, Wv = (np.asarray(a, dtype=np.float32) for a in (Wq, Wk, Wv))
    k_mask = np.asarray(k_mask)
    use_kmask = not bool(np.all(k_mask == 1))

    nc = _get_program(use_kmask)
    in_maps = _host_prep(input_val, Wq, Wk, Wv, k_mask, use_kmask)
    res = bu.run_bass_kernel_spmd(nc, in_maps, core_ids=list(range(N_CORES)))
    return _unshard(res.results)


def kernel_traced(input_val, Wq, Wk, Wv, k_mask, **trace_kwargs):
    """Like kernel() but returns (output, BassKernelResults) with tracing on."""
    import concourse.bass_utils as bu

    input_val = np.asarray(input_val)
    k_mask = np.asarray(k_mask)
    use_kmask = not bool(np.all(k_mask == 1))
    nc = _get_program(use_kmask)
    in_maps = _host_prep(input_val, np.asarray(Wq, dtype=np.float32),
                         np.asarray(Wk, dtype=np.float32),
                         np.asarray(Wv, dtype=np.float32), k_mask, use_kmask)
    res = bu.run_bass_kernel_spmd(nc, in_maps, core_ids=list(range(N_CORES)),
                                  trace=True, **trace_kwargs)
    return _unshard(res.results), res
